# revision 38
# baseline (speedup 1.0000x reference)
"""Trainium2 Bass kernel for the water-network leak MSE model.

Math (reference):
    net(s)   = base[idx_s] + MLP(idx_s)                    (idx_s in [0,1024))
    y        = net*onehot(idx) @ M^T + demand              demand[:, 2j] = D[:, j]
    q        = y @ inv
    hL       = sign(q) * K * |q|^1.852,  K = 10.667 C^-1.852 d^-4.871 L
    H        = (supply - hL) @ inv^T
    d_leak   = Cd*a*sqrt(2g) * (onehot @ M^T) * sqrt(relu(H))
    out      = mean((q @ A0^T - demand - d_leak)^2)

Device strategy (8 cores, data-parallel over samples, 2048 samples/core):
  All sample-independent weight transforms are folded on the host:
    PM  = inv^T M   (so q = net * PM[:, idx] + D @ inv_even),
    AM  = A0' PM    (so q @ A0'^T = net * AM[:, idx] + D @ A0invF),
    A0invF = A0' inv_even^T - [I;0]  (the -I folds the demand subtraction),
  with the per-pipe net table pre-multiplied into PM/AM columns, the
  Hazen-Williams coefficient folded into q itself (q' = K^{1/1.852} q, so
  hL = q'|q'|^0.852 needs no per-pipe scaling on device), and c0 folded
  into the M columns. Node rows are permuted even-first. The per-sample
  row gather of [PM.T | c0*M.T | AM.T] is done ON HOST (numpy fancy
  indexing) and shipped as a per-chunk input — the on-device Q7 gather
  costs ~12us of library load plus ~5us/chunk and gates the pipeline.

  v3 pipeline: chunks are software-pipelined 3 deep so every engine's
  FIFO has step-start-ready work: at step t, chunk a=t runs stage A
  (q matmuls + PSUM drain + |q'|), chunk b=t-1 runs stage B (single-call
  Ln/Exp power chain + hl), chunk d=t-2 runs stages C+E (H matmuls into
  one 4-bank PSUM tile, one-call relu, DVE bit-trick sqrt
  (i>>1)+0x1FC0, d_leak assembly, residual matmuls + drains), and chunk
  s=t-3 gets its square+accumulate first thing on ACT. hl(d) is ready a
  full step before the H matmuls need it, so the PE stream
  [qmm(a) x16, hmm(d) x32, rmm(d) x8] never waits mid-step and stays in
  the fast P-state. PSUM: qpool 2 + hp 4 + rpool 2 = 8 banks.
  Each core returns [128, NCH] partial sums of squares; host reduces.
"""

import math

import numpy as np
import ml_dtypes

P = 128
N_CORES = 8
S_TOTAL = 16384
SC = S_TOTAL // N_CORES  # samples per core
CH = 512                 # samples per chunk
NCH = SC // CH           # chunks per core
N_NODES = 512
N_PIPES = 1024
N_DEM = 256
G_ACC = 9.80665

BF16 = ml_dtypes.bfloat16

_MODULE_CACHE: dict = {}


def _build_module():
    import concourse.bacc as bacc
    import concourse.mybir as mybir
    import concourse.tile as tile

    f32 = mybir.dt.float32
    bf16 = mybir.dt.bfloat16
    i16 = mybir.dt.int16
    AF = mybir.ActivationFunctionType
    OP = mybir.AluOpType

    nc = bacc.Bacc(trn_type="TRN2", target_bir_lowering=False, debug=False)

    # All our activations (Relu/Square/Ln/Exp) live in the
    # natural_log_exp_and_others table set, but the table-load pass maps each
    # func to the first set containing it, ping-ponging between sets. Strip
    # our funcs from every other set so the pass converges on the shared set.
    import types as _types
    from concourse.hw_specs import get_activation_tables as _gat
    import bass_rust as _bass_rust

    _OURS = {AF.Abs, AF.Relu, AF.Square, AF.Ln, AF.Exp, AF.Identity, AF.Copy,
             AF.Sign, AF.MemsetZero}

    def _patched_act_table_loads(self):
        has_activation = any(
            isinstance(i, mybir.InstActivation)
            for b in self.main_func.blocks
            for i in b.instructions
        )
        if not has_activation:
            return
        tables = []
        for name, fns in _gat(self.m.arch).items():
            if name != "natural_log_exp_and_others":
                fns = fns - _OURS
            tables.append((name, fns))
        _bass_rust.insert_act_table_loads(self, tables)

    nc.insert_act_table_loads = _types.MethodType(_patched_act_table_loads, nc)

    # invev split by output block so the first q matmuls start after 0.25MB
    inveva_d = nc.dram_tensor("inveva", [P, 8 * P], bf16, kind="ExternalInput").ap()
    invevb_d = nc.dram_tensor("invevb", [P, 8 * P], bf16, kind="ExternalInput").ap()
    invpt_d = nc.dram_tensor("invpt", [P, 32 * P], bf16, kind="ExternalInput").ap()
    a0inv_d = nc.dram_tensor("a0inv", [P, 8 * P], bf16, kind="ExternalInput").ap()
    # per-chunk D blocks (both K-halves) so chunk 0's load is one small DMA
    dt_ds = [
        nc.dram_tensor(f"dt{c}", [P, 2 * CH], bf16, kind="ExternalInput").ap()
        for c in range(NCH)
    ]
    hsup_d = nc.dram_tensor("hsup", [P, 4], f32, kind="ExternalInput").ap()
    # host-gathered per-sample aux rows, split by use so the q-side half
    # can load first: gq blocks 0-7: PMn.T; gm blocks 0-3: c0*Mp.T,
    # 4-7: AMn.T
    gq_ds = [
        nc.dram_tensor(f"gq{c}", [P, 8, CH], bf16, kind="ExternalInput").ap()
        for c in range(NCH)
    ]
    gm_ds = [
        nc.dram_tensor(f"gm{c}", [P, 8, CH], bf16, kind="ExternalInput").ap()
        for c in range(NCH)
    ]
    bias_d = nc.dram_tensor("biases", [P, 2], f32, kind="ExternalInput").ap()
    ident_d = nc.dram_tensor("ident", [P, P], bf16, kind="ExternalInput").ap()
    out_d = nc.dram_tensor("out_stats", [P, NCH + 1], f32, kind="ExternalOutput").ap()

    with tile.TileContext(nc) as tc:
        with (
            tc.tile_pool(name="const", bufs=1) as cpool,
            tc.tile_pool(name="work", bufs=1) as wpool,
            tc.tile_pool(name="small", bufs=3) as spool,
            tc.tile_pool(name="qps", bufs=2, space="PSUM") as qpool,
            tc.tile_pool(name="hps", bufs=1, space="PSUM") as hpool,
            tc.tile_pool(name="rps", bufs=2, space="PSUM") as rpool,
        ):
            # PE warm-up: ~4.5us of dummy matmuls on zeros during the input
            # DMA wait flips the HAM clock-gate to 8/8 before real work
            zw = cpool.tile([P, CH], bf16, tag="zw")
            nc.vector.memset(zw, 0)
            hpw = hpool.tile([P, 2, CH], f32, tag="hp", bufs=2)
            for _ in range(10):
                nc.tensor.matmul(
                    hpw[:, 0, :], zw[:, 0:P], zw, start=True, stop=True
                )

            # input loads, earliest-needed first (qmm only needs dt0+inveva;
            # the big gather loads must not block them on the DMA queue)
            dt0 = cpool.tile_from(dt_ds[0], name="dt0")
            inveva = cpool.tile_from(inveva_d)
            gq0 = cpool.tile_from(gq_ds[0], name="gq0")
            invevb = cpool.tile_from(invevb_d)
            biases = cpool.tile_from(bias_d)
            dt1 = cpool.tile_from(dt_ds[1], name="dt1")
            invpt = cpool.tile_from(invpt_d)
            gq1 = cpool.tile_from(gq_ds[1], name="gq1")
            hsup = cpool.tile_from(hsup_d)
            a0inv = cpool.tile_from(a0inv_d)
            ident = cpool.tile_from(ident_d)
            dt2 = cpool.tile_from(dt_ds[2], name="dt2")
            gm0 = cpool.tile_from(gm_ds[0], name="gm0")
            gq2 = cpool.tile_from(gq_ds[2], name="gq2")
            dt3 = cpool.tile_from(dt_ds[3], name="dt3")
            gm1 = cpool.tile_from(gm_ds[1], name="gm1")
            gq3 = cpool.tile_from(gq_ds[3], name="gq3")
            gm2 = cpool.tile_from(gm_ds[2], name="gm2")
            gm3 = cpool.tile_from(gm_ds[3], name="gm3")
            gqs = [gq0, gq1, gq2, gq3]
            gms = [gm0, gm1, gm2, gm3]
            dts = [dt0, dt1, dt2, dt3]
            stats = cpool.tile([P, NCH + 1], f32, tag="stats")

            W = 8 * CH  # q-side width per chunk (1024 pipes on 128 parts)

            qsbs, absqs, hls, rls, sqs, r_alls = {}, {}, {}, {}, {}, {}

            for t in range(NCH + 2):
                c_a = t          # stage A: q matmuls + drain + |q'|
                c_b = t - 1      # stage B: power chain + hl
                c_d = t - 2      # stages C/E: H, sqrt, residual
                c_s = t - 3      # square+accumulate

                # ---- A(t): q' = D @ inv_even' + net*PM[:, idx]
                # |q'| runs on the otherwise-idle GPSIMD: it feeds next
                # step's Ln, so its latency is off the critical path
                if c_a < NCH:
                    gd = gqs[c_a]
                    dtc = dts[c_a]
                    qsb = wpool.tile([P, W], bf16, tag="qsb", bufs=2)
                    for pc in range(8):
                        ive = inveva if pc < 4 else invevb
                        pco = pc % 4
                        qp = qpool.tile([P, CH], f32, tag="qp")
                        nc.tensor.matmul(
                            qp,
                            ive[:, (0 * 4 + pco) * P:(0 * 4 + pco + 1) * P],
                            dtc[:, 0:CH],
                            start=True, stop=False,
                        )
                        nc.tensor.matmul(
                            qp,
                            ive[:, (1 * 4 + pco) * P:(1 * 4 + pco + 1) * P],
                            dtc[:, CH:2 * CH],
                            start=False, stop=True,
                        )
                        nc.vector.tensor_tensor(
                            qsb[:, pc * CH:(pc + 1) * CH], qp, gd[:, pc, :],
                            OP.add,
                        )
                        if c_a == 0 and pc == 3:
                            # fill path: chunk 0's first-half power chain
                            # starts as soon as blocks 0-3 are drained
                            absq = wpool.tile([P, W], bf16, tag="absq", bufs=2)
                            lne = wpool.tile([P, W], bf16, tag="lne", bufs=1)
                            e_t = wpool.tile([P, W], bf16, tag="e_t", bufs=1)
                            hl = wpool.tile([P, W], bf16, tag="hl", bufs=2)
                            Hw = W // 2
                            nc.vector.tensor_scalar(
                                absq[:, 0:Hw].bitcast(i16),
                                qsb[:, 0:Hw].bitcast(i16),
                                0x7FFF, None, OP.bitwise_and,
                            )
                            nc.scalar.activation(
                                lne[:, 0:Hw], absq[:, 0:Hw], AF.Ln,
                                bias=biases[:, 0:1],
                            )
                            nc.scalar.activation(
                                e_t[:, 0:Hw], lne[:, 0:Hw], AF.Exp, scale=0.852
                            )
                            nc.vector.tensor_tensor(
                                hl[:, 0:Hw], qsb[:, 0:Hw], e_t[:, 0:Hw],
                                OP.mult,
                            )
                            absqs[0] = absq
                            hls[0] = hl
                            lnes0, e_ts0 = lne, e_t
                    if c_a == 0:
                        Hw = W // 2
                        nc.vector.tensor_scalar(
                            absqs[0][:, Hw:W].bitcast(i16),
                            qsb[:, Hw:W].bitcast(i16),
                            0x7FFF, None, OP.bitwise_and,
                        )
                    else:
                        absq = wpool.tile([P, W], bf16, tag="absq", bufs=2)
                        nc.vector.tensor_scalar(
                            absq.bitcast(i16), qsb.bitcast(i16),
                            0x7FFF, None, OP.bitwise_and,
                        )
                        absqs[c_a] = absq
                    qsbs[c_a] = qsb

                # ---- B(t-1): e = |q'|^0.852, hl = q' * e
                if 0 <= c_b < NCH:
                    if c_b == 0:
                        # second half of chunk 0's power chain
                        Hw = W // 2
                        absq, hl = absqs.pop(0), hls[0]
                        nc.scalar.activation(
                            lnes0[:, Hw:W], absq[:, Hw:W], AF.Ln,
                            bias=biases[:, 0:1],
                        )
                        nc.scalar.activation(
                            e_ts0[:, Hw:W], lnes0[:, Hw:W], AF.Exp, scale=0.852
                        )
                        nc.vector.tensor_tensor(
                            hl[:, Hw:W], qsbs.pop(0)[:, Hw:W],
                            e_ts0[:, Hw:W], OP.mult,
                        )
                    else:
                        lne = wpool.tile([P, W], bf16, tag="lne", bufs=1)
                        e_t = wpool.tile([P, W], bf16, tag="e_t", bufs=1)
                        hl = wpool.tile([P, W], bf16, tag="hl", bufs=2)
                        nc.scalar.activation(
                            lne, absqs.pop(c_b), AF.Ln, bias=biases[:, 0:1]
                        )
                        nc.scalar.activation(e_t, lne, AF.Exp, scale=0.852)
                        nc.vector.tensor_tensor(
                            hl, qsbs.pop(c_b), e_t, OP.mult
                        )
                        hls[c_b] = hl

                # ---- C+E(t-2): H matmuls, then per-bank
                #      relu -> bit-sqrt -> d_leak -> residual so the four
                #      bank chains pipeline across ACT/DVE/PE
                if 0 <= c_d < NCH:
                    hl = hls.pop(c_d)
                    gm = gms[c_d]
                    dtc = dts[c_d]
                    rl = wpool.tile([P, 4, CH], bf16, tag="rl", bufs=2)
                    sq = wpool.tile([P, 4, CH], bf16, tag="sq", bufs=1)
                    r_all = wpool.tile([P, 4 * CH], bf16, tag="r_all", bufs=2)
                    for pr in range(2):
                        # H matmuls for this bank pair: own 2-bank PSUM tile
                        # so next chunk's pair-0 only waits this pair's relu
                        hp = hpool.tile([P, 2, CH], f32, tag="hp", bufs=2)
                        for j, n_ in enumerate((2 * pr, 2 * pr + 1)):
                            if c_d == 0:
                                # fill path: K-halves so chunk 0 starts on
                                # the first half of hl before the second
                                for kh in range(2):
                                    for kc in range(4 * kh, 4 * kh + 4):
                                        nc.tensor.matmul(
                                            hp[:, j, :],
                                            invpt[:, (kc * 4 + n_) * P:
                                                  (kc * 4 + n_ + 1) * P],
                                            hl[:, kc * CH:(kc + 1) * CH],
                                            start=(kc == 0), stop=(kc == 7),
                                        )
                            else:
                                for kc in range(8):
                                    nc.tensor.matmul(
                                        hp[:, j, :],
                                        invpt[:, (kc * 4 + n_) * P:
                                              (kc * 4 + n_ + 1) * P],
                                        hl[:, kc * CH:(kc + 1) * CH],
                                        start=(kc == 0), stop=(kc == 7),
                                    )
                        # residual D-part + PE-injected gathered AM term
                        rps = []
                        for n_ in (2 * pr, 2 * pr + 1):
                            rp = rpool.tile([P, CH], f32, tag="rp")
                            nc.tensor.matmul(
                                rp,
                                a0inv[:, (0 * 4 + n_) * P:(0 * 4 + n_ + 1) * P],
                                dtc[:, 0:CH],
                                start=True, stop=False,
                            )
                            nc.tensor.matmul(
                                rp,
                                a0inv[:, (1 * 4 + n_) * P:(1 * 4 + n_ + 1) * P],
                                dtc[:, CH:2 * CH],
                                start=False, stop=False,
                            )
                            nc.tensor.matmul(
                                rp, ident, gm[:, 4 + n_, :],
                                start=False, stop=True,
                            )
                            rps.append(rp)
                        for j, n_ in enumerate((2 * pr, 2 * pr + 1)):
                            nc.scalar.activation(
                                rl[:, n_, :], hp[:, j, :], AF.Relu,
                                bias=hsup[:, n_:n_ + 1], scale=-1.0,
                            )
                        psl = slice(2 * pr, 2 * pr + 2)
                        # sq = sqrt(rl) via bf16 bit trick (i>>1)+0x1FC0;
                        # rl==0 maps to ~1e-19. c0 is folded into gm.
                        nc.vector.tensor_scalar(
                            sq[:, psl, :].bitcast(i16), rl[:, psl, :].bitcast(i16),
                            1, None, OP.logical_shift_right,
                        )
                        nc.vector.tensor_scalar(
                            sq[:, psl, :].bitcast(i16), sq[:, psl, :].bitcast(i16),
                            0x1FC0, None, OP.add,
                        )
                        dl = spool.tile([P, 2, CH], bf16, tag="dl")
                        nc.vector.tensor_tensor(
                            dl, gm[:, psl, :], sq[:, psl, :], OP.mult
                        )
                        # r = (D-part + AM) - d_leak
                        for j, n_ in enumerate((2 * pr, 2 * pr + 1)):
                            nc.vector.tensor_tensor(
                                r_all[:, n_ * CH:(n_ + 1) * CH], rps[j],
                                dl[:, j, :], OP.subtract,
                            )
                        if c_d == NCH - 1:
                            # drain path: square this bank pair immediately
                            # so only pair 1's chain trails the last matmul
                            scr = wpool.tile([P, 4 * CH], bf16, tag="scr")
                            nc.scalar.activation(
                                scr[:, 0:2 * CH],
                                r_all[:, 2 * pr * CH:(2 * pr + 2) * CH],
                                AF.Square,
                                accum_out=stats[:, NCH - 1 + pr:NCH + pr],
                            )
                            nc.sync.dma_start(
                                out_d[:, NCH - 1 + pr:NCH + pr],
                                stats[:, NCH - 1 + pr:NCH + pr],
                            )
                    r_alls[c_d] = r_all

                # ---- ACT tail: square-accumulate of chunk t-3 (the last
                #      chunk is squared per bank pair inside its CE stage)
                if 0 <= c_s < NCH - 1:
                    scr = wpool.tile([P, 4 * CH], bf16, tag="scr")
                    nc.scalar.activation(
                        scr, r_alls.pop(c_s), AF.Square,
                        accum_out=stats[:, c_s:c_s + 1],
                    )
                    nc.sync.dma_start(
                        out_d[:, c_s:c_s + 1], stats[:, c_s:c_s + 1]
                    )

    nc.compile()
    return nc


def _host_prep(inputs):
    D = np.ascontiguousarray(np.asarray(inputs["D"], np.float32))
    leak = np.asarray(inputs["leak_id"]).reshape(-1).astype(np.int64)
    A0 = np.asarray(inputs["A0"], np.float32)
    inv = np.asarray(inputs["inv"], np.float32)
    M = np.asarray(inputs["M"], np.float32)
    supply = np.asarray(inputs["supply"], np.float32)
    L = np.asarray(inputs["L"], np.float32)
    d = np.asarray(inputs["d"], np.float32)
    C = np.asarray(inputs["C"], np.float32)
    a = float(np.asarray(inputs["a"]))
    Cd = float(np.asarray(inputs["Cd"]))
    W1 = np.asarray(inputs["W1"], np.float32)
    b1 = np.asarray(inputs["b1"], np.float32)
    W2 = np.asarray(inputs["W2"], np.float32)
    b2 = np.asarray(inputs["b2"], np.float32)
    W3 = np.asarray(inputs["W3"], np.float32)
    b3 = np.asarray(inputs["b3"], np.float32)
    base = np.asarray(inputs["base"], np.float32)

    # per-pipe net table (memoized MLP over the 1024 possible leak ids)
    ids = np.arange(N_PIPES, dtype=np.float32)[:, None]
    h = np.tanh(ids @ W1 + b1)
    h = np.tanh(h @ W2 + b2)
    table = base + (h @ W3 + b3)[:, 0]

    perm = np.concatenate([np.arange(0, N_NODES, 2), np.arange(1, N_NODES, 2)])
    Mp = M[perm]
    invp = inv[perm]
    inv_ev = invp[:N_DEM]  # rows of inv at even node indices

    K = 10.667 * C**-1.852 * d**-4.871 * L
    k1 = K ** (1.0 / 1.852)  # fold into q so hL = q'|q'|^0.852

    c0 = Cd * a * math.sqrt(2.0 * G_ACC)

    PM = inv.T @ M                        # [1024p, 1024t]
    PMn = (PM * table[None, :]) * k1[:, None]
    A0p = A0[perm]
    AMn = (A0p @ PM) * table[None, :]     # [512n, 1024t]
    # -I folds the demand subtraction (even node rows come first in perm)
    A0invF = A0p @ inv_ev.T               # [512n, 256j]
    A0invF[:N_DEM] -= np.eye(N_DEM, dtype=np.float32)

    # [1024 rows, 2048]: per-pipe aux row, gathered per sample on host
    maux = np.concatenate([PMn.T, c0 * Mp.T, AMn.T], axis=1).astype(BF16)

    def blocks(mat, kb, mb):
        # [kb*128, mb*128] -> [128, kb*mb*128], block b = kc*mb + mc
        out = np.empty((P, kb * mb * P), np.float32)
        for kc in range(kb):
            for mc in range(mb):
                b = kc * mb + mc
                out[:, b * P:(b + 1) * P] = mat[
                    kc * P:(kc + 1) * P, mc * P:(mc + 1) * P
                ]
        return out

    invev_f = blocks(inv_ev * k1[None, :], 2, 8).astype(BF16)
    # split by output block pc: a = pc 0-3, b = pc 4-7, each [k0 x4 | k1 x4]
    iva = np.concatenate([invev_f[:, (0 * 8 + pc) * P:(0 * 8 + pc + 1) * P]
                          for pc in range(4)] +
                         [invev_f[:, (1 * 8 + pc) * P:(1 * 8 + pc + 1) * P]
                          for pc in range(4)], axis=1)
    ivb = np.concatenate([invev_f[:, (0 * 8 + pc) * P:(0 * 8 + pc + 1) * P]
                          for pc in range(4, 8)] +
                         [invev_f[:, (1 * 8 + pc) * P:(1 * 8 + pc + 1) * P]
                          for pc in range(4, 8)], axis=1)
    invpt_l = blocks(invp.T, 8, 4).astype(BF16)
    a0inv_l = blocks(A0invF.T, 2, 4).astype(BF16)

    hsup_l = np.ascontiguousarray((invp @ supply).reshape(4, P).T).astype(np.float32)

    dts = []
    gd_all = []
    for c in range(N_CORES):
        Dc = D[c * SC:(c + 1) * SC]  # [2048, 256]
        DT = np.ascontiguousarray(Dc.T).astype(BF16)  # [256, 2048]
        dts.append([
            np.ascontiguousarray(np.concatenate(
                [DT[:P, sc * CH:(sc + 1) * CH], DT[P:, sc * CH:(sc + 1) * CH]],
                axis=1,
            ))
            for sc in range(NCH)
        ])
        lc = leak[c * SC:(c + 1) * SC]
        per_chunk = []
        for sc in range(NCH):
            rows = maux[lc[sc * CH:(sc + 1) * CH]]        # [CH, 2048] bf16
            g = rows.reshape(CH, 16, P).transpose(2, 1, 0)  # [P, 16, CH]
            per_chunk.append(
                (np.ascontiguousarray(g[:, :8]), np.ascontiguousarray(g[:, 8:]))
            )
        gd_all.append(per_chunk)

    shared = {
        "inveva": np.ascontiguousarray(iva),
        "invevb": np.ascontiguousarray(ivb),
        "invpt": invpt_l,
        "a0inv": a0inv_l,
        "hsup": hsup_l,
        "ident": np.eye(P, dtype=np.float32).astype(BF16),
    }
    return shared, dts, gd_all


LAST_RESULTS = None


def kernel(**inputs) -> np.ndarray:
    global LAST_RESULTS
    from concourse.bass_utils import run_bass_kernel_spmd

    shared, dts, gd_all = _host_prep(inputs)

    if "nc" not in _MODULE_CACHE:
        _MODULE_CACHE["nc"] = _build_module()
    nc = _MODULE_CACHE["nc"]
    bias_arr = np.zeros((P, 2), np.float32)
    bias_arr[:, 0] = 1e-35

    in_maps = []
    for c in range(N_CORES):
        m = dict(shared)
        m["biases"] = bias_arr
        for sc_ in range(NCH):
            m[f"dt{sc_}"] = dts[c][sc_]
            m[f"gq{sc_}"] = gd_all[c][sc_][0]
            m[f"gm{sc_}"] = gd_all[c][sc_][1]
        in_maps.append(m)

    import os

    res = run_bass_kernel_spmd(
        nc,
        in_maps,
        core_ids=list(range(N_CORES)),
        trace=bool(os.environ.get("BASS_TRACE")),
    )
    LAST_RESULTS = res

    total = 0.0
    for r in res.results:
        total += float(r["out_stats"].astype(np.float64).sum())
    return np.float32(total / (S_TOTAL * N_NODES))
